# revision 2
# baseline (speedup 1.0000x reference)
"""Bass/Trainium2 kernel for batched masked-Kabsch RMSD (nn_Coords2RMSD).

Strategy (per NeuronCore, SPMD across 8 cores):
  - Host sorts batch rows by num_atoms into 4 size classes (quartiles), zeroes
    padded atoms, casts to bf16, and lays the data out TRANSPOSED: SBUF
    partitions = atom-within-chunk (chunks of 128 atoms), free dim =
    (component, chunk, batch-row), grouped into DMA groups of <=4 chunks so
    loads pipeline with compute. All per-row reductions become PE matmuls
    against a ones vector (sum over partitions), nearly free on the PE.
  - Per group: DVE computes 8 of the 9 products x_i*y_j via two broadcast-AP
    bf16 tensor_tensor ops (2x DVE mode); Pool computes the 9th product and
    one square component; Act squares the other five components. PE reduces
    products/squares/sums into 21 PSUM columns per class.
  - Final stage on [128, K] fp32 tiles: centered covariance, eigenvalues of
    C^T C via charpoly identities (p2 = |KK|_F^2 - 3q^2,
    det(KK - qI) = det(C)^2 - q(9q^2 - |KK|_F^2)/2 + 2q^3) and the
    trigonometric method, Kabsch det sign, RMSD.
"""

import numpy as np

import concourse.bass as bass
import concourse.mybir as mybir
from concourse.tile import TileContext, ScopedClock

F32 = mybir.dt.float32
BF16 = mybir.dt.bfloat16
OP = mybir.AluOpType
AF = mybir.ActivationFunctionType
AX = mybir.AxisListType

N_CORES = 8
ROWS = 128
GROUP = 4  # chunks per DMA/compute group
ACT_SQ = 5  # components 0..4 squared on Act, component 5 on Pool


# ---------------------------------------------------------------------------
# TileContext tail patch: this walrus build accepts at most ONE sync-wait
# command per instruction and no sem-eq waits, so the stock drain + EVSEM
# butterfly fails codegen. Emit a ge-wait-only tail instead.
# ---------------------------------------------------------------------------
def _patched_drain_and_barrier(self, tick_clock, wait_clock):
    nc = self.nc
    dummy = nc.gpsimd.nop()
    wait_clock.add_sem_waits(dummy.ins, ScopedClock({None: tick_clock.global_clock}))
    waits = list(dummy.ins.sync_info.on_wait) if dummy.ins.sync_info else []
    if dummy.ins.sync_info:
        dummy.ins.sync_info = mybir.SyncInfo(on_wait=[], on_update=[])

    bsem = nc.alloc_semaphore(f"tail_bsem_{nc.next_id()}")
    dsem = nc.alloc_semaphore(f"tail_dsem_{nc.next_id()}")
    n_eng = 0
    for eng in nc.engines.values():
        eng.drain()
        eng.sem_inc(bsem, 1)
        n_eng += 1
    nc.gpsimd.wait_ge(bsem, n_eng)
    for w in waits:
        n = nc.gpsimd.nop()
        n.ins.sync_info = mybir.SyncInfo(on_wait=[w], on_update=[])
    nc.gpsimd.sem_inc(dsem, 1)
    for eng in nc.engines.values():
        if eng is not nc.gpsimd:
            eng.wait_ge(dsem, 1)

    popped = nc._tile_sem_poison_stack.pop()
    assert popped is self._sem_poison
    nc.clear_and_free_semaphores(list(self.sems.allocated().values()))
    nc.gpsimd.sem_clear(bsem)
    nc.gpsimd.sem_clear(dsem)


def install_tile_patch():
    TileContext._drain_and_barrier = _patched_drain_and_barrier


# ---------------------------------------------------------------------------
# BIR post-pass: this walrus build accepts at most one sync-wait command per
# instruction (none on Drain). Tile's sem-assigner can attach several, so
# split extras onto same-engine NoOps inserted just before the instruction.
# ---------------------------------------------------------------------------
_orig_to_json_bytes = bass.Bass.to_json_bytes


def _split_multiwait_json(self) -> bytes:
    import json

    raw = _orig_to_json_bytes(self)
    m = json.loads(raw)
    ctr = 0
    changed = False
    for f in m.get("functions", []):
        for blk in f.get("blocks", []):
            insts = blk.get("instructions", [])
            out = []
            for inst in insts:
                si = inst.get("sync_info")
                ow = (si or {}).get("on_wait") or []
                opc = str(inst.get("opcode", inst.get("type", "")))
                limit = 0 if opc == "Drain" else 1
                if len(ow) > limit:
                    keep = ow[len(ow) - limit :] if limit else []
                    moved = ow[: len(ow) - limit] if limit else ow
                    for w in moved:
                        ctr += 1
                        out.append(
                            {
                                "debug": inst.get("debug", 0),
                                "engine": inst["engine"],
                                "ins": [],
                                "name": f"WS-{ctr}-{inst['name']}",
                                "opcode": "NoOp",
                                "outs": [],
                                "sync_info": {"on_update": [], "on_wait": [w]},
                            }
                        )
                    si["on_wait"] = keep
                    changed = True
                out.append(inst)
            blk["instructions"] = out
    if not changed:
        return raw
    return json.dumps(m).encode()


bass.Bass.to_json_bytes = _split_multiwait_json


# ---------------------------------------------------------------------------
# Final math emitter on [128, K]-wide fp32 tiles.
# Stats column layout (per class, 21 used of 24):
#   0-8   Rxy(i,j) = sum_n x_i y_j   (pair index m = 3i+j)
#   9-11  Sx_i
#   12-14 Sy_j
#   15-17 sum x_i^2
#   18-20 sum y_j^2
# ---------------------------------------------------------------------------
class _FM:
    def __init__(self, nc, pool, K):
        self.nc = nc
        self.pool = pool
        self.K = K
        self.n = 0
        self._consts = {}

    def const_col(self, val):
        val = float(val)
        if val in self._consts:
            return self._consts[val]
        i = len(self._consts)
        t = self.pool.tile([ROWS, 1], F32, tag=f"fmc{i}", name=f"fmc{i}")
        self.nc.gpsimd.memset(t[:], val)
        self._consts[val] = t[:]
        return t[:]

    def t(self, w=1):
        self.n += 1
        return self.pool.tile([ROWS, w * self.K], F32, tag=f"fm{self.n}", name=f"fm{self.n}")

    def _out_like(self, a):
        return self.t(a.free_size() // self.K)[:]

    def tt(self, a, b, op, out=None, eng=None):
        o = out if out is not None else self._out_like(a)
        (eng or self.nc.vector).tensor_tensor(o, a, b, op)
        return o

    def mul(self, a, b, out=None, eng=None):
        return self.tt(a, b, OP.mult, out, eng)

    def add(self, a, b, out=None, eng=None):
        return self.tt(a, b, OP.add, out, eng)

    def sub(self, a, b, out=None, eng=None):
        return self.tt(a, b, OP.subtract, out, eng)

    def ts(self, a, s, op, out=None, eng=None):
        o = out if out is not None else self._out_like(a)
        (eng or self.nc.vector).tensor_scalar(o, a, float(s), None, op)
        return o

    def ts2(self, a, s1, s2, op0, op1, out=None, eng=None):
        o = out if out is not None else self._out_like(a)
        (eng or self.nc.vector).tensor_scalar(o, a, float(s1), float(s2), op0, op1)
        return o

    def stt(self, a, s, b, op0, op1, out=None, eng=None):
        o = out if out is not None else self._out_like(a)
        (eng or self.nc.vector).scalar_tensor_tensor(o, a, float(s), b, op0, op1)
        return o

    def act(self, a, func, bias=0.0, scale=1.0, out=None):
        o = out if out is not None else self._out_like(a)
        if isinstance(bias, float) and bias not in (0.0, 1.0) and func != AF.Copy:
            bias = self.const_col(bias)
        self.nc.scalar.activation(o, a, func, bias=bias, scale=scale)
        return o

    def recip(self, a, out=None):
        o = out if out is not None else self._out_like(a)
        self.nc.vector.reciprocal(o, a)
        return o

    def red(self, in_ap, out=None, eng=None):
        """Sum over the last AP dim; in_ap must be [128, K, cnt]."""
        o = out if out is not None else self.t()[:]
        (eng or self.nc.vector).tensor_reduce(o, in_ap, AX.X, OP.add)
        return o


def _emit_final_math(nc, fm, stats, meta_ap, out_ap, K):
    """stats: [128, 24*K] fp32 SBUF AP, element (s, k) at column s*K + k."""
    gp = nc.gpsimd

    def S(s, w=1):
        return stats[:, s * K : (s + w) * K]

    def view(ap, a):
        return ap.rearrange("p (a k) -> p a k", a=a)

    def kview(ap, a):
        return ap.rearrange("p (a k) -> p k a", a=a)

    def view33(ap):
        return ap.rearrange("p (a b k) -> p a b k", a=3, b=3)

    rn = fm.recip(meta_ap)

    # --- centered covariance C (pair-major [128, 9K]) ---
    # SS/SSn on Pool: their inputs (sums) are ready before the last products,
    # so they complete during the bulk phase and C starts immediately after.
    sx = view(S(9, 3), 3).unsqueeze(2).broadcast_to([ROWS, 3, 3, K])
    sy = view(S(12, 3), 3).unsqueeze(1).broadcast_to([ROWS, 3, 3, K])
    SS = fm.t(9)
    gp.tensor_tensor(view33(SS[:]), sx, sy, OP.mult)
    rn9 = rn.unsqueeze(1).broadcast_to([ROWS, 9, K])
    SSn = fm.t(9)
    gp.tensor_tensor(view(SSn[:], 9), view(SS[:], 9), rn9, OP.mult)
    C = fm.t(9)
    nc.vector.tensor_tensor(C[:], S(0, 9), SSn[:], OP.subtract)

    def Cs(m, w=1):
        return C[:, m * K : (m + w) * K]

    # --- g = gx + gy (centered): Pool products now, DVE reductions emitted
    # late (below) so they don't delay the critical chain on the in-order
    # DVE queue ---
    S2 = fm.mul(S(9, 6), S(9, 6), eng=gp)  # [128, 6K]

    # --- KK = C^T C, [128, 9K] pair-major (a, b) ---
    KK = fm.t(9)
    tmp1 = fm.t(9)
    tmp2 = fm.t(9)
    for i, dst, eng in ((0, KK, nc.vector), (1, tmp1, gp), (2, tmp2, nc.vector)):
        ca = view(Cs(3 * i, 3), 3).unsqueeze(2).broadcast_to([ROWS, 3, 3, K])
        cb = view(Cs(3 * i, 3), 3).unsqueeze(1).broadcast_to([ROWS, 3, 3, K])
        eng.tensor_tensor(view33(dst[:]), ca, cb, OP.mult)
    fm.add(KK[:], tmp2[:], out=KK[:])
    fm.add(KK[:], tmp1[:], out=KK[:])

    # --- det(C) (DVE; on the detKq critical path) ---
    O12 = fm.t(9)  # C_1a * C_2b outer [128, 9K]
    c1 = view(Cs(3, 3), 3).unsqueeze(2).broadcast_to([ROWS, 3, 3, K])
    c2 = view(Cs(6, 3), 3).unsqueeze(1).broadcast_to([ROWS, 3, 3, K])
    nc.vector.tensor_tensor(view33(O12[:]), c1, c2, OP.mult)

    def Os(m):
        return O12[:, m * K : (m + 1) * K]

    M = fm.t(3)
    fm.sub(Os(5), Os(7), out=M[:, 0:K])            # C11 C22 - C12 C21
    fm.sub(Os(6), Os(2), out=M[:, K : 2 * K])      # -(C10 C22 - C12 C20)
    fm.sub(Os(1), Os(3), out=M[:, 2 * K : 3 * K])  # C10 C21 - C11 C20
    TD = fm.mul(Cs(0, 3), M[:])
    detC = fm.red(kview(TD, 3))

    # --- q = tr(KK)/3 = |C|_F^2 / 3 (available before KK completes) ---
    C2t = fm.mul(C[:], C[:])
    trK = fm.red(kview(C2t, 9))
    q = fm.ts(trK, 1.0 / 3.0, OP.mult)
    qq = fm.mul(q, q)
    KK2 = fm.mul(KK[:], KK[:])
    sall = fm.red(kview(KK2, 9))

    # --- p2 = sall - 3 q^2 ; p = sqrt(max(p2/6, tiny)) ---
    p2 = fm.stt(qq, -3.0, sall, OP.mult, OP.add)
    p2c = fm.ts2(p2, 1.0 / 6.0, 1e-30, OP.mult, OP.max)
    p_ = fm.act(p2c, AF.Sqrt)

    # --- half-detKq = 0.5 det(KK - qI) = 0.5 detC^2 - 1.25 q^3 + 0.25 q sall ---
    q3 = fm.mul(qq, q)
    dC2h = fm.stt(detC, 0.5, detC, OP.mult, OP.mult)
    t1 = fm.mul(q, sall)
    t2 = fm.stt(q3, -1.25, dC2h, OP.mult, OP.add)
    detKqh = fm.stt(t1, 0.25, t2, OP.mult, OP.add)

    # --- r = clamp(0.5 detKq / p^3, [-1, 1]) ---
    p3 = fm.mul(p2c, p_)
    rp3 = fm.recip(p3)
    r0 = fm.mul(detKqh, rp3)
    r = fm.ts2(r0, 1.0, -1.0, OP.min, OP.max)

    # g reductions, off the critical chain
    s2s = fm.red(kview(S2, 6))
    graw = fm.red(kview(S(15, 6), 6))
    gcor = fm.mul(s2s, rn, eng=gp)
    g = fm.sub(graw, gcor, eng=gp)

    # --- acos(r) via Hastings: acos(|r|) = sqrt(1-|r|) * poly(|r|), err<2e-4;
    #     acos(r) = acos(|r|)(1-2neg) + pi neg ---
    rabs = fm.stt(r, -1.0, r, OP.mult, OP.max)
    sq1 = fm.act(rabs, AF.Sqrt, bias=1.0, scale=-1.0)     # sqrt(1-|r|)
    # Estrin: poly = (a0 + a1 x) + x^2 (a2 + a3 x)  [runs under the Sqrt act]
    u1 = fm.ts2(rabs, -0.2121144, 1.5707288, OP.mult, OP.add)
    u2 = fm.ts2(rabs, -0.0187293, 0.0742610, OP.mult, OP.add)
    x2 = fm.mul(rabs, rabs)
    v = fm.mul(x2, u2)
    po = fm.add(v, u1)
    rneg = fm.ts(r, 0.0, OP.is_lt, eng=gp)                # parallel branch
    s2c = fm.ts2(rneg, -2.0, 1.0, OP.mult, OP.add, eng=gp)  # 1 - 2 neg
    pim = fm.ts(rneg, float(np.pi), OP.mult, eng=gp)        # pi neg
    acA = fm.mul(sq1, po)                                 # acos(|r|)
    tac = fm.mul(acA, s2c)
    acr = fm.add(tac, pim)                                # acos(r)

    # cos(phi) = sin(phi + pi/2); cos(phi + 2pi/3) = -sin(5pi/6 - phi)
    # (keep Sin args in [pi/2, 5pi/6] -- the HW table dislikes negatives)
    CC = fm.t(2)
    fm.act(acr, AF.Sin, bias=float(np.pi / 2), scale=1.0 / 3.0, out=CC[:, 0:K])
    fm.act(acr, AF.Sin, bias=float(5 * np.pi / 6), scale=-1.0 / 3.0, out=CC[:, K : 2 * K])

    # --- eigenvalues: L = [l1, l3, l2]; l1 = q + 2p c1, l3 = q - 2p c3m ---
    L = fm.t(3)
    P2 = fm.t(2)
    p2x = fm.ts(p_, 2.0, OP.mult, out=P2[:, 0:K])
    fm.ts(p_, -2.0, OP.mult, out=P2[:, K : 2 * K])
    qb = q.unsqueeze(1).broadcast_to([ROWS, 2, K])
    tpc = fm.t(2)
    nc.vector.tensor_tensor(view(tpc[:], 2), view(P2[:], 2), view(CC[:], 2), OP.mult)
    nc.vector.tensor_tensor(view(L[:, 0 : 2 * K], 2), qb, view(tpc[:], 2), OP.add)
    t5 = fm.stt(q, 3.0, L[:, 0:K], OP.mult, OP.subtract)
    fm.sub(t5, L[:, K : 2 * K], out=L[:, 2 * K : 3 * K])  # l2
    Lc = fm.ts(L[:], 0.0, OP.max)
    sv = fm.act(Lc, AF.Sqrt)  # [s1, s3, s2]

    # --- Kabsch sign (on Pool), trace term, rmsd ---
    dneg = fm.ts(detC, 0.0, OP.is_lt, eng=gp)
    d = fm.ts2(dneg, -2.0, 1.0, OP.mult, OP.add, eng=gp)
    fm.mul(d, sv[:, K : 2 * K], out=sv[:, K : 2 * K])  # d*s3 in place
    tr = fm.red(kview(sv, 3))
    diff = fm.stt(tr, -2.0, g, OP.mult, OP.add)
    msdc = fm.stt(diff, 0.0, rn, OP.max, OP.mult)  # max(diff,0)*rn
    fm.act(msdc, AF.Sqrt, out=out_ap)


# ---------------------------------------------------------------------------
# Program builder
# ---------------------------------------------------------------------------
def _class_groups(ck, first_small=False):
    """Split ck chunks into groups of <= GROUP chunks. With first_small, the
    leading groups are 1 and 2 chunks so the compute pipeline fills fast."""
    out = []
    c = 0
    if first_small and ck > 2:
        out.append((0, 2))
        c = 2
    while c < ck:
        g = min(GROUP, ck - c)
        if ck - c - g == 1:  # avoid a trailing 1-chunk group
            g -= 1
        out.append((c, g))
        c += g
    return out


def _default_class_order(cks):
    """Mid-size class first (small pipeline fill), biggest classes in the
    middle, smallest last (short tail before the final math)."""
    co = sorted(range(len(cks)), key=lambda k: -cks[k])
    if len(co) > 2:
        co = [co[2]] + co[:2] + co[3:]
    return co


def build_program(cks, cfg=None):
    """cks: per-class chunk counts (atom capacity = ck*128). Returns nc."""
    cfg = cfg or {}
    K = len(cks)
    class_order = cfg.get("class_order", _default_class_order(cks))

    install_tile_patch()
    nc = bass.Bass()

    # dram layout: class-major contiguous blocks [6, ck, 128] in class_order
    offs = {}
    col = 0
    for k in class_order:
        offs[k] = col
        col += 6 * cks[k] * 128
    total_cols = col

    xy_d = nc.dram_tensor("xy", [ROWS, total_cols], BF16, kind="ExternalInput")
    meta_d = nc.dram_tensor("meta", [ROWS, K], F32, kind="ExternalInput")
    out_d = nc.dram_tensor("out", [ROWS, K], F32, kind="ExternalOutput")
    dbg_d = None
    if cfg.get("debug_stats"):
        dbg_d = nc.dram_tensor("dbg", [ROWS, 21 * K], F32, kind="ExternalOutput")

    with TileContext(nc) as tc:
        with (
            tc.tile_pool(name="const", bufs=1) as constp,
            tc.tile_pool(name="xy", bufs=2) as xyp,
            tc.tile_pool(name="prod", bufs=2) as prodp,
            tc.tile_pool(name="sq", bufs=2) as sqp,
            tc.tile_pool(name="psum", bufs=2, space="PSUM") as psp,
            tc.tile_pool(name="stats", bufs=1) as statp,
        ):
            first = True
            meta_t = None
            ones_t = None
            stats = None
            fm = None
            pending_copies = []

            def flush_copies():
                for k_, st_ in pending_copies:
                    nc.vector.tensor_copy(
                        stats[:, k_ : k_ + 15 * K : K], st_[:, 0:15]
                    )
                    nc.vector.tensor_copy(
                        stats[:, 15 * K + k_ : 15 * K + k_ + 6 * K : K],
                        st_[:, 15:21],
                    )
                pending_copies.clear()

            def late_init():
                # emitted after the very first DMA so it hits the wire first
                nonlocal meta_t, ones_t, stats, fm
                meta_t = constp.tile([ROWS, K], F32, name="meta_t")
                nc.scalar.dma_start(out=meta_t[:], in_=meta_d[:])
                ones_t = constp.tile([ROWS, 1], BF16, name="ones_t")
                nc.gpsimd.memset(ones_t[:], 1.0)
                stats = statp.tile([ROWS, 24 * K], F32, name="stats")
                fm = _FM(nc, statp, K)

            for j, k in enumerate(class_order):
                ck = cks[k]
                KB = ck * 128
                last_class = j == K - 1
                # pool takes pair (1,2) for the first wq chunks (load balance)
                wq = 0 if last_class else int(round(cfg.get("wq_frac", 0.3) * ck))
                xy_t = xyp.tile([ROWS, 6 * KB], BF16, tag="xy", name="xy_t")
                p_t = prodp.tile([ROWS, 9 * KB], BF16, tag="prod", name="p_t")
                pl_t = prodp.tile([ROWS, 3 * KB], BF16, tag="pool", name="pl_t")
                sq_t = sqp.tile([ROWS, ACT_SQ * KB], BF16, tag="sq", name="sq_t")
                zc = xy_t[:].rearrange("p (a c b) -> p a c b", a=6, b=128)
                xc = zc[:, 0:3]
                yc = zc[:, 3:6]
                pc = p_t[:, 0 : 9 * KB].rearrange(
                    "p (i j c b) -> p i j c b", i=3, j=3, b=128
                )
                sqc = sq_t[:].rearrange("p (a c b) -> p a c b", a=ACT_SQ, b=128)

                for c0, gc in _class_groups(ck, first_small=(j == 0)):
                    CB = gc * 128
                    b0, b1 = c0 * 128, (c0 + gc) * 128
                    # group DMA: 6 comps x gc chunks, strided in the class blk
                    nc.sync.dma_start(
                        out=xy_t[:].rearrange("p (a c) -> p a c", a=6)[:, :, b0:b1],
                        in_=xy_d[
                            :, offs[k] : offs[k] + 6 * KB
                        ].rearrange("p (a c) -> p a c", a=6)[:, :, b0:b1],
                    )
                    if first:
                        late_init()
                        first = False

                    # products: (i, j<2) on DVE always
                    nc.vector.tensor_tensor(
                        pc[:, :, 0:2, c0 : c0 + gc, :],
                        xc[:, :, c0 : c0 + gc, :]
                        .unsqueeze(2)
                        .broadcast_to([ROWS, 3, 2, gc, 128]),
                        yc[:, 0:2, c0 : c0 + gc, :]
                        .unsqueeze(1)
                        .broadcast_to([ROWS, 3, 2, gc, 128]),
                        OP.mult,
                    )
                    # (0,2) always DVE; (1,2) DVE only for chunks >= wq
                    i_hi = 1 if c0 + gc <= wq else 2
                    nc.vector.tensor_tensor(
                        pc[:, 0:i_hi, 2, c0 : c0 + gc, :],
                        xc[:, 0:i_hi, c0 : c0 + gc, :],
                        yc[:, 2, c0 : c0 + gc, :]
                        .unsqueeze(1)
                        .broadcast_to([ROWS, i_hi, gc, 128]),
                        OP.mult,
                    )
                    if i_hi == 2 and c0 < wq:
                        # straddling group: redo nothing; (1,2) chunks < wq on pool
                        pass
                    if c0 < wq:
                        ph = min(wq, c0 + gc)
                        nc.gpsimd.tensor_tensor(
                            pl_t[:, 2 * KB : 3 * KB].rearrange(
                                "p (c b) -> p c b", b=128
                            )[:, c0:ph, :],
                            xc[:, 1, c0:ph, :],
                            yc[:, 2, c0:ph, :],
                            OP.mult,
                        )
                    if last_class:
                        # keep Pool's lag off the final-math critical path:
                        # (2,2) product on DVE, only y2^2 on Pool
                        nc.vector.tensor_tensor(
                            pc[:, 2, 2, c0 : c0 + gc, :],
                            xc[:, 2, c0 : c0 + gc, :],
                            yc[:, 2, c0 : c0 + gc, :],
                            OP.mult,
                        )
                        nc.gpsimd.tensor_tensor(
                            pl_t[:, 0 : 2 * KB].rearrange(
                                "p (a c b) -> p a c b", a=2, b=128
                            )[:, 1, c0 : c0 + gc, :],
                            yc[:, 2, c0 : c0 + gc, :],
                            yc[:, 2, c0 : c0 + gc, :],
                            OP.mult,
                        )
                    else:
                        # Pool: (x2*y2, y2*y2) -> pl_t blocks 0, 1
                        nc.gpsimd.tensor_tensor(
                            pl_t[:, 0 : 2 * KB].rearrange(
                                "p (a c b) -> p a c b", a=2, b=128
                            )[:, :, c0 : c0 + gc, :],
                            zc[:, 2:6:3, c0 : c0 + gc, :],
                            yc[:, 2, c0 : c0 + gc, :]
                            .unsqueeze(1)
                            .broadcast_to([ROWS, 2, gc, 128]),
                            OP.mult,
                        )
                    # squares comps 0..4 on Act
                    nc.scalar.activation(
                        sqc[:, :, c0 : c0 + gc, :],
                        zc[:, 0:ACT_SQ, c0 : c0 + gc, :],
                        AF.Square,
                    )

                # previous class's stats copies: their deps are met by now,
                # so they slot into the DVE stream without stalling it
                flush_copies()

                # class-end PE reductions: per column start..stop contiguous
                # (a start marks the whole 2KB PSUM zero-region, so groups in
                # one bank must not interleave)
                st_t = psp.tile([ROWS, 24], F32, tag=f"st{j}", name="st_t")
                srcs = (
                    [(xy_t, a, 9 + a) for a in range(6)]
                    + [(p_t, m, m) for m in range(8)]
                    + ([(p_t, 8, 8)] if j == K - 1 else [(pl_t, 0, 8)])
                    + [(sq_t, a, 15 + a) for a in range(ACT_SQ)]
                    + [(pl_t, 1, 15 + ACT_SQ)]
                )
                for src, blk, col_i in srcs:
                    for c in range(ck):
                        s_, b_ = src, blk
                        if col_i == 5 and c < wq:  # pool-computed (1,2) chunks
                            s_, b_ = pl_t, 2
                        base = b_ * KB
                        nc.tensor.matmul(
                            st_t[:, col_i : col_i + 1],
                            s_[:, base + c * 128 : base + (c + 1) * 128],
                            ones_t[:],
                            start=(c == 0),
                            stop=(c == ck - 1),
                        )
                pending_copies.append((k, st_t))

            flush_copies()
            if dbg_d is not None:
                dbg_t = statp.tile([ROWS, 21 * K], F32, name="dbg_t")
                nc.vector.tensor_copy(dbg_t[:], stats[:, 0 : 21 * K])
                nc.sync.dma_start(out=dbg_d[:], in_=dbg_t[:])
            out_t = statp.tile([ROWS, K], F32, name="out_t")
            _emit_final_math(nc, fm, stats[:], meta_t[:], out_t[:], K)
            nc.sync.dma_start(out=out_d[:], in_=out_t[:])

    return nc


# ---------------------------------------------------------------------------
# Host side
# ---------------------------------------------------------------------------
def plan_shards(num_atoms, n_classes=4):
    B = num_atoms.shape[0]
    assert B % (N_CORES * ROWS) == 0
    assert n_classes == B // (N_CORES * ROWS)
    order = np.argsort(num_atoms, kind="stable")
    na_sorted = num_atoms[order]
    rows_per_class = N_CORES * ROWS
    cks = []
    for k in range(n_classes):
        mx = int(na_sorted[(k + 1) * rows_per_class - 1])
        cks.append((mx + 127) // 128)
    return order, cks


def shard_inputs(coords_input, coords_target, num_atoms, order, cks, nmax, class_order):
    import ml_dtypes

    K = len(cks)
    rows_per_class = N_CORES * ROWS
    bf16 = ml_dtypes.bfloat16

    # per-class transposed tensors built once, then sliced into dram layout
    in_maps = [dict() for _ in range(N_CORES)]
    core_row_idx = [[] for _ in range(N_CORES)]
    per_core_blocks = [[] for _ in range(N_CORES)]
    meta = [np.empty((ROWS, K), dtype=np.float32) for _ in range(N_CORES)]

    for j, k in enumerate(class_order):
        ck = cks[k]
        na_cap = ck * 128
        for c in range(N_CORES):
            rows = order[k * rows_per_class + c * ROWS : k * rows_per_class + (c + 1) * ROWS]
            core_row_idx[c].append((k, rows))
            na = num_atoms[rows].astype(np.int64)
            meta[c][:, k] = na.astype(np.float32)
            mask = (np.arange(na_cap)[None, :] < na[:, None]).astype(bf16)
            comp = []
            for arr in (coords_input, coords_target):
                v = arr[rows].reshape(ROWS, nmax, 3)[:, :na_cap, :].astype(bf16)
                v = v * mask[:, :, None]
                # [b, (chunk, p), i] -> [p, i, chunk, b]
                comp.append(v.reshape(ROWS, ck, 128, 3).transpose(2, 3, 1, 0))
            # z: [128, 6, ck, 128b] -> one contiguous class-major block
            z = np.concatenate(comp, axis=1)
            per_core_blocks[c].append(np.ascontiguousarray(z).reshape(128, -1))

    for c in range(N_CORES):
        in_maps[c] = {
            "xy": np.concatenate(per_core_blocks[c], axis=1),
            "meta": meta[c],
        }
    return in_maps, core_row_idx


def unshard_outputs(results, core_row_idx, B):
    out = np.empty(B, dtype=np.float32)
    for c in range(N_CORES):
        o = np.asarray(results[c]["out"], dtype=np.float32)  # [ROWS, K]
        for k, rows in core_row_idx[c]:
            out[rows] = o[:, k]
    return out


# ---------------------------------------------------------------------------
# Entry point
# ---------------------------------------------------------------------------
_PROG_CACHE = {}


def _get_program(cks):
    key = tuple(cks)
    if key not in _PROG_CACHE:
        _PROG_CACHE[key] = build_program(list(cks))
    return _PROG_CACHE[key]


def kernel(coords_input, coords_target, num_atoms):
    from concourse.bass_utils import run_bass_kernel_spmd

    x = np.ascontiguousarray(np.asarray(coords_input, dtype=np.float32))
    y = np.ascontiguousarray(np.asarray(coords_target, dtype=np.float32))
    na = np.asarray(num_atoms).astype(np.int64)
    B, ncols = x.shape
    nmax = ncols // 3
    K = B // (N_CORES * ROWS)
    assert B == N_CORES * ROWS * K, f"unsupported batch {B}"

    order, cks = plan_shards(na, n_classes=K)
    class_order = _default_class_order(cks)
    in_maps, core_row_idx = shard_inputs(x, y, na, order, cks, nmax, class_order)
    nc = _get_program(cks)
    res = run_bass_kernel_spmd(nc, in_maps, core_ids=list(range(N_CORES)))
    out = unshard_outputs(res.results, core_row_idx, B)
    return out.astype(np.float32)
